# revision 63
# baseline (speedup 1.0000x reference)
"""Trainium2 Bass kernel for nn_CausalTransitionModel (GNN message passing).

Data-parallel over the batch: each of 8 NeuronCores owns 64 graphs.
Edge MLP layer 1 is decomposed as concat(x_i,x_j)@W1 = U_i + V_j with
U = x@W1[:D], V = x@W1[D:] computed per-node; per-edge tiles are built
feature-major via PE matmuls against static 0/1 selection matrices.
LayerNorm mean is folded into W2 on the host; the variance sum runs as a
broadcasting ones-matrix matmul so 1/std lands on all 128 partitions in
one PSUM tile (recip on DVE, sqrt on ACT, no separate broadcast matmul).
segment_sum is a strided free-axis DVE reduce straight into a bf16 agg
(15 consecutive edges per receiver).  All matmuls run in bf16 with fp32
PSUM accumulation.
"""

import numpy as np

import concourse.bass as bass
import concourse.bacc as bacc
import concourse.mybir as mybir
from concourse import tile
from concourse.bass_utils import run_bass_kernel_spmd

# Problem shapes (hardcoded per contract).
B, K, D, H, A = 512, 16, 128, 512, 8
EPS = 1e-5
NCORES = 8
BG = B // NCORES          # graphs per core = 64
NPC = BG * K              # nodes per core = 1024
EPG = K * (K - 1)         # edges per graph = 240
TILE_G = 2                # graphs per edge tile
TILE_E = TILE_G * EPG     # edges per tile = 480
NT = BG // TILE_G         # edge tiles per core = 32
FCH = H // 128            # feature chunks = 4
SEG = K - 1               # segment length = 15
# node-phase parts (lo, width, last feeding edge tile); the tail parts are
# narrow and interleaved so their serial chains pipeline against each other
NODE_PARTS = [(0, 320, 9), (320, 320, 19), (640, 192, 25), (832, 96, 28),
              (928, 96, 31)]

BF16 = mybir.dt.bfloat16
F32 = mybir.dt.float32

_prog_cache: dict = {}


def _canonical_edge_index() -> np.ndarray:
    pairs = np.array(
        [(i, j) for i in range(K) for j in range(K) if i != j], dtype=np.int64
    )
    offs = (np.arange(B, dtype=np.int64) * K)[:, None, None]
    return (pairs[None] + offs).reshape(-1, 2).T


def _seluv_matrix() -> np.ndarray:
    """[128, TILE_E] 0/1 selection: rows 0:32 pick U_i (receiver), rows 32:64
    pick V_j (sender) for the 480 edges of a graph pair; rows 64:128 repeat
    the pattern so pairs stacked at partition base 64 can use the same
    constant (PE row groups follow the operand's base partition)."""
    sel = np.zeros((64, TILE_E), np.float32)
    for e in range(TILE_E):
        g_loc = e // EPG
        w = e % EPG
        i = w // SEG
        jj = w % SEG
        j = jj if jj < i else jj + 1
        sel[g_loc * K + i, e] = 1.0
        sel[32 + g_loc * K + j, e] = 1.0
    return np.concatenate([sel, sel], axis=0)


def _chunk_major(w: np.ndarray) -> np.ndarray:
    """[K_in, M] -> [128, (K_in//128)*M] with slice [:, k*M+m] = w[k*128+p, m]."""
    kin, m = w.shape
    nk = kin // 128
    return np.ascontiguousarray(
        w.reshape(nk, 128, m).transpose(1, 0, 2).reshape(128, nk * m)
    )


def _per_part(b: np.ndarray) -> np.ndarray:
    """[H] -> [128, H//128] fp32 per-partition bias layout (chunk c at col c)."""
    return np.ascontiguousarray(b.reshape(-1, 128).T.astype(np.float32))


def _build_program(edge_fast: bool, node_fast: bool, repeat: int | None = None,
                   no_ln: bool = False, pipeline: int = 4, bz: bool = True,
                   b2p_zero: bool = True):
    nc = bacc.Bacc("TRN2", target_bir_lowering=False, debug=False,
                   num_devices=NCORES)

    def din(name, shape, dt=BF16):
        return nc.dram_tensor(name, shape, dt, kind="ExternalInput").ap()

    xT = din("xT", [128, NPC])                  # states, feature-major
    actT = din("actT", [A, NPC])                # one-hot action, transposed
    w1r = din("w1r", [128, H])
    w1c = din("w1c", [128, H])
    w2p = din("w2p", [128, FCH * H])            # mean-folded, chunk-major
    wn1x = din("wn1x", [128, H])
    wn1a = din("wn1a", [A, H])
    wn1g = din("wn1g", [128, FCH * H])
    wn2p = din("wn2p", [128, FCH * H])
    wn3 = din("wn3", [128, FCH * D])
    seluv = din("seluv", [128, TILE_E])
    ones128 = din("ones128", [128, 128])        # colsum-broadcast lhsT
    b1 = din("b1", [128, FCH], F32)
    b2p = din("b2p", [128, FCH], F32)
    g2 = din("g2", [128, FCH], F32)
    bln2 = din("bln2", [128, FCH], F32)
    bn1 = din("bn1", [128, FCH], F32)
    bn2p = din("bn2p", [128, FCH], F32)
    gn = din("gn", [128, FCH], F32)
    blnn = din("blnn", [128, FCH], F32)
    bn3 = din("bn3", [128, 1], F32)

    out = nc.dram_tensor("out", [128, NPC], F32, kind="ExternalOutput").ap()

    AF = mybir.ActivationFunctionType
    OP = mybir.AluOpType

    with tile.TileContext(nc) as tc:
        cpool = tc.alloc_tile_pool(name="const", bufs=1)
        wpool6 = tc.alloc_tile_pool(name="work6", bufs=8)
        p_t0 = tc.alloc_tile_pool(name="p_t0", bufs=5)
        p_sq = tc.alloc_tile_pool(name="p_sq", bufs=4)
        p_a2 = tc.alloc_tile_pool(name="p_a2", bufs=4)
        wpool3 = tc.alloc_tile_pool(name="work3", bufs=5)
        ppool = tc.alloc_tile_pool(name="psum", bufs=3, space="PSUM")
        zpool = tc.alloc_tile_pool(name="psum_z", bufs=2, space="PSUM")
        spool = tc.alloc_tile_pool(name="psum_s", bufs=1, space="PSUM")

        def load(ap, tag):
            t = cpool.tile(list(ap.shape), ap.dtype, tag=tag)
            nc.sync.dma_start(t[:], ap)
            return t

        c_xT = cpool.tile([128, NPC], BF16, tag="xT")
        nc.sync.dma_start(c_xT[:, :256], xT[:, :256])
        nc.sync.dma_start(c_xT[:, 256:], xT[:, 256:])
        c_w1r = load(w1r, "w1r")
        c_w1c = load(w1c, "w1c")
        c_seluv = load(seluv, "seluv")
        c_w2p = load(w2p, "w2p")
        c_ones = load(ones128, "ones128")
        c_actT = load(actT, "actT")
        c_wn1x = load(wn1x, "wn1x")
        c_wn1a = load(wn1a, "wn1a")
        c_wn1g = load(wn1g, "wn1g")
        c_wn2p = load(wn2p, "wn2p")
        c_wn3 = load(wn3, "wn3")
        c_b1 = load(b1, "b1")
        c_b2p = load(b2p, "b2p")
        c_g2 = load(g2, "g2")
        c_bln2 = load(bln2, "bln2")
        c_bn1 = load(bn1, "bn1")
        c_bn2p = load(bn2p, "bn2p")
        c_gn = load(gn, "gn")
        c_blnn = load(blnn, "blnn")
        c_bn3 = load(bn3, "bn3")

        # Persistent SBUF state (allocated once).
        uv = cpool.tile([128, 16 * H], BF16, tag="uv")    # stacked UV pairs
        agg = cpool.tile([128, FCH * NPC], BF16, tag="agg")  # [h%128, c*1024+n]
        out_sb = cpool.tile([128, NPC], F32, tag="out_sb")

        def body():
            # ---- UV blocks, stacked per pair of graphs ----
            # uv block blk holds pairs 2*blk (partitions 0:32 U / 32:64 V)
            # and 2*blk+1 (64:96 U / 96:128 V); col-tiled matmul outputs.
            # Blocks 0-2 run up front; the rest interleave with edge tiles
            # (block b is emitted 3+ blocks before its z1 consumers).
            def uv_block(blk):
                ps = ppool.tile([128, H], F32, tag="z2")
                for sub in range(4):
                    pr, half = divmod(sub, 2)
                    nodes = (2 * blk + pr) * 32
                    nc.tensor.matmul(
                        ps[32 * sub:32 * (sub + 1), :],
                        c_xT[:, nodes: nodes + 32],
                        (c_w1r if half == 0 else c_w1c)[:],
                        start=True, stop=True, tile_position=(0, 32 * sub))
                dst = uv[:, blk * H:(blk + 1) * H]
                if blk % 2 == 0:
                    nc.scalar.copy(dst, ps[:])
                else:
                    nc.vector.tensor_copy(dst, ps[:])

            for blk in range(3):
                uv_block(blk)

            # ---- Phase 2: edge tiles, 4-deep software pipeline ----
            # A1(t): z1 gather -> a1 relu (runs one iter ahead of A2 so the
            #        z2 matmuls never wait on ACT)
            # A2(t-1): z2 -> t0 -> sq
            # S1(t-d1): ssq broadcast matmul + recip (DVE)
            # S2(t-d2): rb = sqrt(H*vinv) (ACT) ; a2 = relu(t0)*rb (Pool)
            # S3(t-d3): segmented reduce into bf16 agg (DVE + Pool halves)
            st = {}

            def stage_a1(t):
                blk, par = t // 2, t % 2
                ubase = blk * H
                suv = c_seluv[64 * par: 64 * (par + 1), :]
                a1 = wpool6.tile([128, FCH * TILE_E], BF16, tag="a1")
                for pr in range(2):
                    z1p = zpool.tile([128, 1024], F32, tag="z1p")
                    for ci in range(2):
                        c = 2 * pr + ci
                        nc.tensor.matmul(
                            z1p[:, ci * 512: ci * 512 + TILE_E],
                            uv[64 * par: 64 * (par + 1),
                               ubase + c * 128: ubase + (c + 1) * 128],
                            suv, start=True, stop=True)
                    if bz:
                        src = z1p[:].rearrange("p (u v) -> p u v", v=512)[:, :, :TILE_E]
                        nc.scalar.activation(
                            a1[:, 2 * pr * TILE_E: 2 * (pr + 1) * TILE_E]
                            .rearrange("p (u v) -> p u v", v=TILE_E),
                            src, AF.Relu)
                    else:
                        for ci in range(2):
                            c = 2 * pr + ci
                            nc.scalar.activation(
                                a1[:, c * TILE_E:(c + 1) * TILE_E],
                                z1p[:, ci * 512: ci * 512 + TILE_E],
                                AF.Relu, bias=c_b1[:, c:c + 1])
                st[t] = {"a1": a1}

            def stage_a2(t):
                d = st[t]
                a1 = d.pop("a1")
                t0 = p_t0.tile([128, FCH * TILE_E], BF16, tag="t0")
                for m in range(FCH):
                    z2 = ppool.tile([128, TILE_E], F32, tag="z2")
                    for k in range(FCH):
                        nc.tensor.matmul(
                            z2[:], c_w2p[:, k * H + m * 128: k * H + (m + 1) * 128],
                            a1[:, k * TILE_E:(k + 1) * TILE_E],
                            start=(k == 0), stop=(k == FCH - 1))
                    nc.scalar.activation(t0[:, m * TILE_E:(m + 1) * TILE_E],
                                         z2[:], AF.Identity,
                                         bias=c_b2p[:, m:m + 1])
                sq = p_sq.tile([128, FCH * TILE_E], BF16, tag="sq")
                nc.vector.tensor_mul(sq[:], t0[:], t0[:])
                # pre-sum the 4 feature chunks pairwise on Pool so the ssq
                # matmul only streams 2 chunks instead of 4
                sqh = p_sq.tile([128, 2 * TILE_E], BF16, tag="sqh")
                nc.gpsimd.tensor_add(sqh[:], sq[:, :2 * TILE_E],
                                     sq[:, 2 * TILE_E:])
                d["t0"] = t0
                d["sqh"] = sqh

            def stage_s1(t):
                if no_ln:
                    return
                d = st[t]
                # ssq broadcast: ones[128,128]^T @ sq_chunk sums features over
                # partitions and lands the per-edge total on ALL partitions.
                # eps is dropped: var ~ O(1) >> 1e-5 for this model.
                sqh = d.pop("sqh")
                ssqb = spool.tile([128, TILE_E], F32, tag="ssqb")
                for m in range(2):
                    nc.tensor.matmul(
                        ssqb[:], c_ones[:],
                        sqh[:, m * TILE_E:(m + 1) * TILE_E],
                        start=(m == 0), stop=(m == 1))
                vinv = wpool3.tile([128, TILE_E], F32, tag="vinv")
                nc.vector.reciprocal_approx_fast(out=vinv[:], in_=ssqb[:])
                d["vinv"] = vinv

            def stage_s2(t):
                d = st[t]
                abig = p_a2.tile([128, FCH * TILE_E], BF16, tag="a2")
                if no_ln:
                    nc.vector.tensor_scalar_max(abig[:], d["t0"][:], 0.0)
                    d["a2"] = abig
                    return
                # rstd = sqrt(H * (1/ssq)), already on all 128 partitions
                rb = wpool3.tile([128, TILE_E], BF16, tag="rb")
                nc.scalar.activation(rb[:], d["vinv"][:], AF.Sqrt,
                                     scale=float(H))
                if edge_fast:
                    # relu(t0)*rstd == relu(t0*rstd) since rstd > 0;
                    # rb broadcasts over chunk pairs via a 0-stride AP
                    rbb = rb[:].rearrange("p (a e) -> p a e", a=1)\
                        .broadcast_to((128, 2, TILE_E))
                    for h in range(2):
                        nc.vector.scalar_tensor_tensor(
                            abig[:, 2 * h * TILE_E: 2 * (h + 1) * TILE_E]
                            .rearrange("p (a e) -> p a e", a=2),
                            d["t0"][:, 2 * h * TILE_E: 2 * (h + 1) * TILE_E]
                            .rearrange("p (a e) -> p a e", a=2),
                            0.0, rbb, OP.max, OP.mult)
                else:
                    for m in range(FCH):
                        a = abig[:, m * TILE_E:(m + 1) * TILE_E]
                        tm = d["t0"][:, m * TILE_E:(m + 1) * TILE_E]
                        u = wpool3.tile([128, TILE_E], BF16, tag="u")
                        nc.vector.tensor_mul(u[:], tm, rb[:])
                        nc.scalar.activation(a, u[:], AF.Relu,
                                             bias=c_bln2[:, m:m + 1],
                                             scale=c_g2[:, m:m + 1])
                d["a2"] = abig

            def stage_s3(t):
                # agg-pre-W3: layer 3 and segment_sum are both linear, so
                # segsum first; W3 is folded into the node layer-1 weights.
                # Pool pre-folds 14 of the 15 segment terms pairwise, DVE
                # reduces the 7 partials and adds the leftover edge.
                d = st.pop(t)
                src = d["a2"][:].rearrange("p (m s e) -> p m s e", s=2 * K,
                                           e=SEG)
                dst = agg[:].rearrange("p (m n) -> p m n", m=FCH)[
                    :, :, 32 * t: 32 * (t + 1)]
                rfold = p_sq.tile([128, FCH * 2 * K * 7], BF16, tag="rfold")
                rf = rfold[:].rearrange("p (m s e) -> p m s e", s=2 * K, e=7)
                with nc.allow_low_precision(reason="segsum of 15 bf16 terms"):
                    nc.gpsimd.tensor_add(rf, src[:, :, :, 0:7],
                                         src[:, :, :, 7:14])
                    nc.vector.tensor_reduce(dst, rf, mybir.AxisListType.X,
                                            OP.add)
                    last = src[:, :, :, 14:15].rearrange(
                        "p m s e -> p m (s e)")
                    nc.gpsimd.tensor_add(dst, dst, last)

            # ---- Node MLP as weavable sub-stages; parts overlap the edge
            # loop as their agg inputs complete. ----
            def node_stages(lo, w):
                nsl = slice(lo, lo + w)
                dn = {"a1n": {}, "a2n": {}}

                def sL1(ms):
                    def f():
                        for m in ms:
                            z = ppool.tile([128, w], F32, tag="z2")
                            nc.tensor.matmul(
                                z[:], c_wn1x[:, m * 128:(m + 1) * 128],
                                c_xT[:, nsl], start=True, stop=False)
                            nc.tensor.matmul(
                                z[:], c_wn1a[:, m * 128:(m + 1) * 128],
                                c_actT[:, nsl], start=False, stop=False)
                            for k in range(FCH):
                                nc.tensor.matmul(
                                    z[:],
                                    c_wn1g[:, k * H + m * 128: k * H + (m + 1) * 128],
                                    agg[:, k * NPC + lo: k * NPC + lo + w],
                                    start=False, stop=(k == FCH - 1))
                            a = wpool6.tile([128, w], BF16, tag="a1")
                            nc.scalar.activation(a[:], z[:], AF.Relu,
                                                 bias=c_bn1[:, m:m + 1])
                            dn["a1n"][m] = a
                    return f

                def sL2(ms):
                    def f():
                        if "t0n" not in dn:
                            t0n = p_t0.tile([128, FCH * w], BF16, tag="t0n")
                            dn["t0n"] = t0n
                        t0n = dn["t0n"]
                        for m in ms:
                            z2 = ppool.tile([128, w], F32, tag="z2")
                            for k in range(FCH):
                                nc.tensor.matmul(
                                    z2[:],
                                    c_wn2p[:, k * H + m * 128: k * H + (m + 1) * 128],
                                    dn["a1n"][k][:], start=(k == 0),
                                    stop=(k == FCH - 1))
                            dstm = t0n[:, m * w:(m + 1) * w]
                            if m < 2:
                                nc.scalar.activation(dstm, z2[:], AF.Identity,
                                                     bias=c_bn2p[:, m:m + 1])
                            else:
                                nc.vector.tensor_scalar_add(
                                    dstm, z2[:], c_bn2p[:, m:m + 1])
                    return f

                def sStats():
                    t0n = dn["t0n"]
                    sqn = p_sq.tile([128, FCH * w], BF16, tag="sqn")
                    nc.vector.tensor_mul(sqn[:], t0n[:], t0n[:])
                    ssqb = spool.tile([128, w], F32, tag="ssqb")
                    for m in range(FCH):
                        nc.tensor.matmul(ssqb[:], c_ones[:],
                                         sqn[:, m * w:(m + 1) * w],
                                         start=(m == 0), stop=(m == FCH - 1))
                    vinv = wpool3.tile([128, w], F32, tag="vinvn")
                    nc.vector.reciprocal_approx_fast(out=vinv[:], in_=ssqb[:])
                    dn["vinv"] = vinv

                def sA2():
                    t0n = dn["t0n"]
                    rb = wpool3.tile([128, w], BF16, tag="rbn")
                    nc.scalar.activation(rb[:], dn["vinv"][:], AF.Sqrt,
                                         scale=float(H))
                    for m in range(FCH):
                        a = p_a2.tile([128, w], BF16, tag="a2n")
                        tm = t0n[:, m * w:(m + 1) * w]
                        if node_fast:
                            nc.vector.scalar_tensor_tensor(
                                a[:], tm, 0.0, rb[:], OP.max, OP.mult)
                        else:
                            u = wpool3.tile([128, w], BF16, tag="u")
                            nc.vector.tensor_mul(u[:], tm, rb[:])
                            nc.scalar.activation(a[:], u[:], AF.Relu,
                                                 bias=c_blnn[:, m:m + 1],
                                                 scale=c_gn[:, m:m + 1])
                        dn["a2n"][m] = a

                def sL3():
                    z3 = ppool.tile([128, w], F32, tag="z2")
                    for k in range(FCH):
                        nc.tensor.matmul(z3[:], c_wn3[:, k * 128:(k + 1) * 128],
                                         dn["a2n"][k][:], start=(k == 0),
                                         stop=(k == FCH - 1))
                    nc.scalar.activation(out_sb[:, nsl], z3[:], AF.Identity,
                                         bias=c_bn3[:, 0:1])
                    nc.sync.dma_start(out[:, nsl], out_sb[:, nsl])

                return [sL1((0, 1)), sL1((2, 3)), sL2((0, 1)), sL2((2, 3)),
                        sStats, sA2, sL3]

            if pipeline == 0:
                for blk in range(3, 16):
                    uv_block(blk)
                for t in range(NT):
                    stage_a1(t)
                    stage_a2(t)
                    stage_s1(t)
                    stage_s2(t)
                    stage_s3(t)
                for lo, w, lastt in NODE_PARTS:
                    for st_fn in node_stages(lo, w):
                        st_fn()
            else:
                dA = 1
                d1 = dA + 1
                d2 = dA + 2
                d3 = dA + 3
                # node part [lo, lo+w) depends on edge tiles lo/32..lastt,
                # whose segsums land by iteration lastt+d3; weave the 7
                # sub-stages right after. The parts shrink toward the end
                # so the unoverlappable tail is small.
                sched = {}
                for lo, w, lastt in NODE_PARTS:
                    base = lastt + 1 + d3
                    for si, fn in enumerate(node_stages(lo, w)):
                        sched.setdefault(base + si, []).append(fn)
                last = max(sched)
                for t in range(max(NT + d3 + 1, last + 1)):
                    # uv block b feeds a1 tiles 2b, 2b+1: emit 3 blocks ahead
                    blk = (t + 6) // 2
                    if t % 2 == 0 and 3 <= blk < 16:
                        uv_block(blk)
                    if t < NT:
                        stage_a1(t)
                    if 0 <= t - dA < NT:
                        stage_a2(t - dA)
                    if 0 <= t - d1 < NT:
                        stage_s1(t - d1)
                    if 0 <= t - d2 < NT:
                        stage_s2(t - d2)
                    if 0 <= t - d3 < NT:
                        stage_s3(t - d3)
                    for fn in sched.pop(t, ()):
                        fn()


        if repeat:
            with tc.For_i(0, repeat, 1):
                body()
        else:
            body()

        spool.release()
        zpool.release()
        ppool.release()
        wpool3.release()
        p_a2.release()
        p_sq.release()
        p_t0.release()
        wpool6.release()
        cpool.release()

    nc.compile()
    return nc


def _get_program(edge_fast: bool, node_fast: bool, repeat: int | None = None,
                 bz: bool = True, b2p_zero: bool = True):
    key = (edge_fast, node_fast, repeat, bz, b2p_zero)
    if key not in _prog_cache:
        _prog_cache[key] = _build_program(edge_fast, node_fast, repeat, bz=bz,
                                          b2p_zero=b2p_zero)
    return _prog_cache[key]


def _numpy_reference(states, action, edge_index, edge_w1, edge_b1, edge_w2,
                     edge_b2, edge_ln_g, edge_ln_b, edge_w3, edge_b3, node_w1,
                     node_b1, node_w2, node_b2, node_ln_g, node_ln_b, node_w3,
                     node_b3):
    def ln(x, g, b):
        m = x.mean(-1, keepdims=True)
        v = x.var(-1, keepdims=True)
        return (x - m) / np.sqrt(v + EPS) * g + b

    Bs, Kn, Dd = states.shape
    node = states.reshape(-1, Dd).astype(np.float32)
    row, col = np.asarray(edge_index[0]), np.asarray(edge_index[1])
    e = np.concatenate([node[row], node[col]], axis=1)
    e = np.maximum(e @ edge_w1 + edge_b1, 0)
    e = np.maximum(ln(e @ edge_w2 + edge_b2, edge_ln_g, edge_ln_b), 0)
    e = e @ edge_w3 + edge_b3
    agg = np.zeros((node.shape[0], e.shape[1]), np.float32)
    np.add.at(agg, row, e)
    act = np.zeros((Bs, A * Kn), np.float32)
    act[np.arange(Bs), np.asarray(action)] = 1.0
    act = act.reshape(-1, A)
    h = np.concatenate([node, act, agg], axis=1)
    h = np.maximum(h @ node_w1 + node_b1, 0)
    h = np.maximum(ln(h @ node_w2 + node_b2, node_ln_g, node_ln_b), 0)
    return (h @ node_w3 + node_b3).reshape(Bs, Kn, -1)


def _prepare_in_maps(states, action, edge_w1, edge_b1, edge_w2, edge_b2,
                     edge_ln_g, edge_ln_b, edge_w3, edge_b3, node_w1, node_b1,
                     node_w2, node_b2, node_ln_g, node_ln_b, node_w3, node_b3):
    bf = mybir.dt.np(BF16)
    f32 = np.float32

    edge_w1 = np.asarray(edge_w1, f32)
    edge_w2 = np.asarray(edge_w2, f32)
    edge_w3 = np.asarray(edge_w3, f32)
    node_w1 = np.asarray(node_w1, f32)
    node_w2 = np.asarray(node_w2, f32)
    node_w3 = np.asarray(node_w3, f32)
    edge_b3 = np.asarray(edge_b3, f32)

    w2p = edge_w2 - edge_w2.mean(axis=1, keepdims=True)
    b2p = np.asarray(edge_b2, f32) - np.asarray(edge_b2, f32).mean()
    wn2p = node_w2 - node_w2.mean(axis=1, keepdims=True)
    bn2p = np.asarray(node_b2, f32) - np.asarray(node_b2, f32).mean()
    # b3e enters every edge message; segment_sum adds it 15x per node -> fold
    # through the agg slot of node_w1 into the node layer-1 bias.
    bn1 = np.asarray(node_b1, f32) + SEG * (edge_b3 @ node_w1[D + A:])

    seluv = _seluv_matrix()
    common = {
        "w1r": edge_w1[:D].astype(bf),
        "w1c": edge_w1[D:].astype(bf),
        "w2p": _chunk_major(w2p).astype(bf),
        "wn1x": node_w1[:D].astype(bf),
        "wn1a": node_w1[D:D + A].astype(bf),
        "wn1g": _chunk_major(edge_w3 @ node_w1[D + A:]).astype(bf),
        "wn2p": _chunk_major(wn2p).astype(bf),
        "wn3": _chunk_major(node_w3).astype(bf),
        "seluv": seluv.astype(bf),
        "ones128": np.ones((128, 128), f32).astype(bf),
        "b1": _per_part(np.asarray(edge_b1, f32)),
        "b2p": _per_part(b2p),
        "g2": _per_part(np.asarray(edge_ln_g, f32)),
        "bln2": _per_part(np.asarray(edge_ln_b, f32)),
        "bn1": _per_part(bn1),
        "bn2p": _per_part(bn2p),
        "gn": _per_part(np.asarray(node_ln_g, f32)),
        "blnn": _per_part(np.asarray(node_ln_b, f32)),
        "bn3": np.asarray(node_b3, f32).reshape(128, 1),
    }

    states = np.asarray(states, f32)
    action = np.asarray(action)
    in_maps = []
    for c in range(NCORES):
        x = states[BG * c:BG * (c + 1)].reshape(NPC, D)
        act_c = np.asarray(action[BG * c:BG * (c + 1)], np.int64)
        actT = np.zeros((A, NPC), f32)
        for bloc in range(BG):
            av = int(act_c[bloc])
            k, a = av // A, av % A
            actT[a, bloc * K + k] = 1.0
        m = dict(common)
        m["xT"] = np.ascontiguousarray(x.T).astype(bf)
        m["actT"] = actT.astype(bf)
        in_maps.append(m)

    edge_fast = bool(np.all(np.asarray(edge_ln_g, f32) == 1.0)
                     and np.all(np.asarray(edge_ln_b, f32) == 0.0))
    node_fast = bool(np.all(np.asarray(node_ln_g, f32) == 1.0)
                     and np.all(np.asarray(node_ln_b, f32) == 0.0))
    bz = bool(np.all(np.asarray(edge_b1, f32) == 0.0))
    b2pz = bool(np.all(b2p == 0.0))
    return in_maps, edge_fast, node_fast, bz, b2pz


def kernel(**inputs) -> np.ndarray:
    states = np.asarray(inputs["states"])
    edge_index = np.asarray(inputs["edge_index"])
    if not np.array_equal(edge_index.astype(np.int64), _canonical_edge_index()):
        return np.asarray(
            _numpy_reference(**{k: np.asarray(v) for k, v in inputs.items()}),
            np.float32)

    in_maps, edge_fast, node_fast, bz, b2pz = _prepare_in_maps(
        states, inputs["action"], inputs["edge_w1"], inputs["edge_b1"],
        inputs["edge_w2"], inputs["edge_b2"], inputs["edge_ln_g"],
        inputs["edge_ln_b"], inputs["edge_w3"], inputs["edge_b3"],
        inputs["node_w1"], inputs["node_b1"], inputs["node_w2"],
        inputs["node_b2"], inputs["node_ln_g"], inputs["node_ln_b"],
        inputs["node_w3"], inputs["node_b3"])

    nc = _get_program(edge_fast, node_fast, bz=bz, b2p_zero=b2pz)
    res = run_bass_kernel_spmd(nc, in_maps, list(range(NCORES)))
    out = np.empty((B, K, D), np.float32)
    for c in range(NCORES):
        out[BG * c:BG * (c + 1)] = (
            res.results[c]["out"].T.reshape(BG, K, D))
    return out
